# revision 32
# baseline (speedup 1.0000x reference)
"""Trainium2 kernel for nn_Graph_41609643163904.

The reference op is a sequential per-cell scatter sweep over a 48x48 grid:
for x in 2..45, y in 2..45 (x outer): read center v, zero it, add v*W[y,x]
to the 5x5 neighborhood.  Every step is linear in the grid, so the whole
sweep is one fixed linear operator M (2304x2304) depending only on the
weights.  We build M on the host, then the device work is a batched matmul
out = in @ M^T, data-parallel over the 8192-sample batch across 8
NeuronCores (1024 samples/core), zero comm.

Device-side structure (v3.2):
  * x-major re-flattening exposes the sweep's causal cone as block
    sparsity: per 128-wide j-tile only a prefix of k-tiles is nonzero
    (188 of 324 blocks).
  * two-tier precision, tuned on the host against an exact product:
      - slow tier: bf16 weights x bf16 activations, fp32 PSUM.
      - fast tier (adjacent k-tile pairs): e4m3 DoubleRow matmuls
        contract 256 rows per pass (~2x PE throughput).  Pairs are
        picked greedily by MEASURED output-error increase (not block
        energy), any alignment, until the simulated relative error hits
        ERR_TARGET.
    The fast tier accumulates in its own PSUM bank (weights pre-scaled
    by SC for e4m3 range); the merge out = psA + psB/SC rides on the
    PSUM->SBUF drain (scalar-engine scaled copy + DVE add).
  * out^T orientation: M' blocks stationary, batch (512-wide moving
    operand) streams through.  Batch shard resident in SBUF; M streams
    through bounded pools in consumption order.
  * DMA spread over 4 rings: M on sync, xb split over scalar/vector,
    stores tiered gpsimd -> scalar -> sync so the final stores drain on
    fast idle rings.  Small tiles are processed last to shorten the
    post-matmul tail.
"""

import os

import numpy as np
import ml_dtypes

SIZE = 48
D = 2
K = 5
N = SIZE * SIZE          # 2304
B = 8192
NCORES = 8
BS = B // NCORES         # 1024 samples per core

P = 128
NK = N // P              # 18 k-tiles
NJ = N // P              # 18 j-tiles
MW = 512                 # moving-operand width (max for fp32 PSUM bank)
NM = BS // MW            # 2 m-tiles per core

ERR_TARGET = 1.80e-2     # budget for quantization error (limit is 2e-2)
SIM_NS = 512             # samples used in the host error simulation

# Structural nonzero k-tile prefix per 128-wide j-tile (x-major layout).
KPREF = tuple(
    min(NK, -(-(SIZE * ((P * (t + 1) - 1) // SIZE + 3)) // P)) for t in range(NJ)
)


def _build_M(weights: np.ndarray) -> np.ndarray:
    """Compose the 1936 per-cell updates into one (N, N) operator, fp64."""
    M = np.eye(N, dtype=np.float64)
    w = weights.astype(np.float64)
    for x in range(D, SIZE - D):
        for y in range(D, SIZE - D):
            c = y * SIZE + x
            wc = w[y, x]
            rc = M[c].copy()
            for dy in range(-D, D + 1):
                r0 = c + dy * SIZE - D
                wrow = wc[dy + D]
                if dy == 0:
                    M[r0:r0 + D] += np.outer(wrow[:D], rc)
                    M[r0 + D + 1:r0 + K] += np.outer(wrow[D + 1:], rc)
                else:
                    M[r0:r0 + K] += np.outer(wrow, rc)
            M[c] = wc[D, D] * rc
    return M


def _xmajor_idx():
    n = np.arange(N)
    return (n % SIZE) * SIZE + n // SIZE


def _q(a, dt):
    return a.astype(dt).astype(np.float32)


def _pick_fast_pairs(Mp: np.ndarray, xP: np.ndarray):
    """Greedy: convert adjacent k-tile pairs to e4m3 DoubleRow, picking the
    pair with the smallest measured output-error increase each round, until
    the simulated total relative error reaches ERR_TARGET.

    Returns (fast: dict t -> list of k0, SC).
    """
    f32 = np.float32
    MT = Mp.T
    cand = []   # (t, k0)
    stats = {t: [] for t in range(NJ)}   # (energy, max_entry)
    for t in range(NJ):
        for k0 in range(KPREF[t] - 1):
            blk = MT[k0 * P:(k0 + 2) * P, t * P:(t + 1) * P]
            stats[t].append((float((blk ** 2).sum()),
                             float(np.abs(blk).max())))
            cand.append((t, k0))
    # Per-tile e4m3 scale sized for the lower-energy half of that tile's
    # candidate pairs (the ones the greedy actually converts); big-entry
    # pairs that would clip get a large measured error and are skipped.
    SC = []
    for t in range(NJ):
        st = sorted(stats[t])
        if not st:
            SC.append(1.0)
            continue
        mx = max(m for _, m in st[:max(1, len(st) // 2)])
        SC.append(float(2.0 ** np.floor(np.log2(240.0 / mx))))

    xs = xP[:SIM_NS].astype(f32)
    xb = _q(xs, ml_dtypes.bfloat16)
    x4 = _q(np.clip(xs, -240, 240), ml_dtypes.float8_e4m3)
    Mf32 = Mp.astype(f32)
    out_exact = xs.astype(np.float64) @ Mp.T
    Mb = _q(Mf32, ml_dtypes.bfloat16)
    err = (xb @ Mb.T).astype(np.float64) - out_exact   # [ns, N]
    den = np.linalg.norm(out_exact)

    # Per-candidate error-delta vectors (confined to j-tile t's columns).
    dvec = {}
    for t, k0 in cand:
        js = slice(t * P, (t + 1) * P)
        ks = slice(k0 * P, (k0 + 2) * P)
        blk = Mf32[js, ks]
        blk_q = _q(np.clip(blk * SC[t], -240, 240),
                   ml_dtypes.float8_e4m3) / SC[t]
        dvec[(t, k0)] = (x4[:, ks] @ blk_q.T
                         - xb[:, ks] @ Mb[js, ks].T).astype(np.float64)

    err_sq = float(np.linalg.norm(err) ** 2)
    fast = {t: [] for t in range(NJ)}
    used = {t: set() for t in range(NJ)}
    alive = dict(dvec)
    while alive:
        best, best_inc, best_d = None, None, None
        for (t, k0), d in alive.items():
            inc = float(2.0 * np.tensordot(err[:, t * P:(t + 1) * P], d)
                        + np.linalg.norm(d) ** 2)
            if best_inc is None or inc < best_inc:
                best, best_inc, best_d = (t, k0), inc, d
        t, k0 = best
        if np.sqrt(max(err_sq + best_inc, 0.0)) / den > ERR_TARGET:
            break
        err_sq += best_inc
        err[:, t * P:(t + 1) * P] += best_d
        fast[t].append(k0)
        used[t].update((k0, k0 + 1))
        alive = {(tt, kk): d for (tt, kk), d in alive.items()
                 if not (tt == t and (kk in used[t] or kk + 1 in used[t]))}
        if (t, k0) in alive:
            del alive[(t, k0)]
    return fast, SC


def _build_device_kernel(slow_ks, fast_k0, ns_off, nq_off, SC,
                         ntot_s, ntot_q):
    import concourse.mybir as mybir
    from concourse import bacc
    from concourse.tile import TileContext

    f32 = mybir.dt.float32
    bf16 = mybir.dt.bfloat16
    f8e4 = mybir.dt.float8e4
    Copy = mybir.ActivationFunctionType.Copy
    DR = mybir.MatmulPerfMode.DoubleRow

    nc = bacc.Bacc()
    xb = nc.dram_tensor("xb", [N, BS], bf16, kind="ExternalInput")
    x4 = nc.dram_tensor("x4", [N, BS], f8e4, kind="ExternalInput")
    ms = nc.dram_tensor("ms", [P, max(ntot_s, 1) * P], bf16,
                        kind="ExternalInput")
    mf = nc.dram_tensor("mf", [P, max(ntot_q, 1) * 2 * P], f8e4,
                        kind="ExternalInput")
    outT = nc.dram_tensor("outT", [N, BS], bf16, kind="ExternalOutput")

    xb_r = xb.rearrange("(k p) m -> k p m", p=P)
    x4_r = x4.rearrange("(k p) m -> p k m", p=P)

    # x4 lives in one contiguous k-major SBUF tile so any adjacent k-tile
    # pair is a valid [p, 2, n] DoubleRow slice; loaded in 5 range DMAs.
    X4_CHUNK = 4

    # Process small tiles 2 and 3 last so the post-matmul drain (merge +
    # store) of the final tile is short; big tiles' drains overlap them.
    order = [0, 1] + list(range(4, NJ)) + [3, 2]

    with TileContext(nc) as tc:
        with (
            tc.tile_pool(name="xpool", bufs=1) as xpool,
            tc.tile_pool(name="mpool", bufs=6) as mpool,
            tc.tile_pool(name="fpool", bufs=6) as fpool,
            tc.tile_pool(name="opool", bufs=3) as opool,
            tc.tile_pool(name="tpool", bufs=4) as tpool,
            tc.tile_pool(name="pspool", bufs=2, space="PSUM") as pspool,
        ):
            xb_tiles = {}
            x4all = xpool.tile([P, NK, BS], f8e4, tag="x4all", name="x4all")
            issued_k = 0
            issued_c = 0

            def issue_x(upto_k):
                nonlocal issued_k, issued_c
                while issued_k < min(upto_k, NK):
                    k = issued_k
                    xt = xpool.tile([P, BS], bf16, tag=f"x{k}", name=f"x{k}")
                    # first two k-tiles ride the sync ring ahead of the M
                    # stream so the first matmul starts sooner.
                    eng = nc.sync if k < 2 else nc.scalar
                    eng.dma_start(out=xt[:], in_=xb_r[k])
                    xb_tiles[k] = xt
                    issued_k += 1
                while issued_c * X4_CHUNK < issued_k:
                    c0 = issued_c * X4_CHUNK
                    c1 = min(c0 + X4_CHUNK, NK)
                    nc.scalar.dma_start(out=x4all[:, c0:c1, :],
                                        in_=x4_r[:, c0:c1, :])
                    issued_c += 1

            for pos, t in enumerate(order):
                sks = slow_ks[t]
                fks = fast_k0[t]
                ns_t, nq_t = len(sks), len(fks)
                if ns_t:
                    mst = mpool.tile([P, ns_t * P], bf16, tag="ms",
                                     name=f"ms{t}")
                    nc.sync.dma_start(
                        out=mst[:],
                        in_=ms[:, ns_off[t] * P:(ns_off[t] + ns_t) * P],
                    )
                if nq_t:
                    mft = fpool.tile([P, nq_t * 2 * P], f8e4, tag="mf",
                                     name=f"mf{t}")
                    nc.sync.dma_start(
                        out=mft[:],
                        in_=mf[:, nq_off[t] * 2 * P:(nq_off[t] + nq_t) * 2 * P],
                    )
                lookahead = [order[p] for p in (pos + 1, pos + 2, pos + 3)
                             if p < NJ]
                issue_x(max([KPREF[t]] + [KPREF[n] for n in lookahead]))

                ot = opool.tile([P, BS], bf16, tag="o", name=f"o{t}")
                psA = psB = None
                if ns_t:
                    psA = {m: pspool.tile([P, MW], f32, tag=f"psA{m}",
                                          name=f"psA{t}_{m}")
                           for m in range(NM)}
                if nq_t:
                    psB = {m: pspool.tile([P, MW], f32, tag=f"psB{m}",
                                          name=f"psB{t}_{m}")
                           for m in range(NM)}

                for i, k in enumerate(sks):
                    for m in range(NM):
                        nc.tensor.matmul(
                            psA[m][:],
                            lhsT=mst[:, i * P:(i + 1) * P],
                            rhs=xb_tiles[k][:, m * MW:(m + 1) * MW],
                            start=(i == 0),
                            stop=(i == ns_t - 1),
                        )
                for q, k0 in enumerate(fks):
                    lhsT = mft[:, q * 2 * P:(q + 1) * 2 * P].rearrange(
                        "p (two j) -> p two j", two=2)
                    for m in range(NM):
                        nc.tensor.matmul(
                            psB[m][:],
                            lhsT=lhsT,
                            rhs=x4all[:, k0:k0 + 2, m * MW:(m + 1) * MW],
                            start=(q == 0),
                            stop=(q == nq_t - 1),
                            perf_mode=DR,
                        )

                for m in range(NM):
                    osl = ot[:, m * MW:(m + 1) * MW]
                    if ns_t and nq_t:
                        tmp = tpool.tile([P, MW], bf16, tag="tmp",
                                         name=f"tmp{t}_{m}")
                        nc.scalar.activation(tmp[:], psB[m][:], Copy,
                                             scale=1.0 / SC[t])
                        nc.vector.tensor_add(osl, psA[m][:], tmp[:])
                    elif ns_t:
                        nc.vector.tensor_copy(osl, psA[m][:])
                    else:
                        nc.scalar.activation(osl, psB[m][:], Copy,
                                             scale=1.0 / SC[t])
                    # Tiered store rings: slow SWDGE early, then the HWDGE
                    # rings as their input streams drain.
                    if pos < 9:
                        st_eng = nc.gpsimd
                    elif pos < 14:
                        st_eng = nc.scalar
                    else:
                        st_eng = nc.sync
                    st_eng.dma_start(
                        out=outT[t * P:(t + 1) * P, m * MW:(m + 1) * MW],
                        in_=osl,
                    )
    if not nc.is_finalized():
        nc.finalize()
    return nc


def kernel(inputs: np.ndarray, weights: np.ndarray) -> np.ndarray:
    from concourse.bass_utils import run_bass_kernel_spmd

    inputs = np.ascontiguousarray(inputs, dtype=np.float32)
    weights = np.ascontiguousarray(weights, dtype=np.float32)

    M = _build_M(weights)
    idx = _xmajor_idx()
    Mp = M[np.ix_(idx, idx)]
    xP = inputs.reshape(B, SIZE, SIZE).transpose(0, 2, 1).reshape(B, N)

    fast, SC = _pick_fast_pairs(Mp, xP)
    if os.environ.get("KERNEL_TRACE"):
        print(f"fast pairs: {sum(len(v) for v in fast.values())} "
              f"SC: {sorted(set(SC))}")

    slow_ks, fast_k0 = [], []
    for t in range(NJ):
        fks = sorted(fast[t])
        in_fast = {k for k0 in fks for k in (k0, k0 + 1)}
        slow_ks.append([k for k in range(KPREF[t]) if k not in in_fast])
        fast_k0.append(fks)

    # Host packing.  ms: bf16 slow blocks, t-major then k.  mf: e4m3 fast
    # pairs, t-major then (pair, two, j).
    MTf = np.ascontiguousarray(Mp.T.astype(np.float32))
    ms_cols, mf_cols = [], []
    ns_off, nq_off = [], []
    ns_tot = nq_tot = 0
    for t in range(NJ):
        ns_off.append(ns_tot)
        nq_off.append(nq_tot)
        js = slice(t * P, (t + 1) * P)
        for k in slow_ks[t]:
            ms_cols.append(MTf[k * P:(k + 1) * P, js])
        for k0 in fast_k0[t]:
            mf_cols.append(MTf[k0 * P:(k0 + 1) * P, js])
            mf_cols.append(MTf[(k0 + 1) * P:(k0 + 2) * P, js])
        ns_tot += len(slow_ks[t])
        nq_tot += len(fast_k0[t])

    ms_packed = (np.concatenate(ms_cols, axis=1) if ms_cols
                 else np.zeros((P, P), np.float32))
    mf_packed = (np.concatenate(mf_cols, axis=1) if mf_cols
                 else np.zeros((P, 2 * P), np.float32))
    sc_cols = np.concatenate(
        [np.full(2 * P, SC[t], np.float32)
         for t in range(NJ) for _ in fast_k0[t]]) if mf_cols else \
        np.ones(2 * P, np.float32)
    ms_arr = ms_packed.astype(ml_dtypes.bfloat16)
    mf_arr = np.clip(mf_packed * sc_cols[None, :], -240,
                     240).astype(ml_dtypes.float8_e4m3)

    xb_full = xP.astype(ml_dtypes.bfloat16)
    x4_full = np.clip(xP, -240, 240).astype(ml_dtypes.float8_e4m3)

    nc = _build_device_kernel(slow_ks, fast_k0, ns_off, nq_off, SC,
                              ns_tot, nq_tot)
    in_maps = [
        {
            "xb": np.ascontiguousarray(xb_full[c * BS:(c + 1) * BS].T),
            "x4": np.ascontiguousarray(x4_full[c * BS:(c + 1) * BS].T),
            "ms": ms_arr,
            "mf": mf_arr,
        }
        for c in range(NCORES)
    ]
    trace = bool(int(os.environ.get("KERNEL_TRACE", "0")))
    res = run_bass_kernel_spmd(
        nc, in_maps, core_ids=list(range(NCORES)), trace=trace
    )
    if trace and res.exec_time_ns is not None:
        print(f"HW exec time: {res.exec_time_ns} ns")
        if res.instructions_and_trace is not None:
            print(f"trace: {res.instructions_and_trace[1]}")

    outP = np.concatenate(
        [res.results[c]["outT"].astype(np.float32).T for c in range(NCORES)],
        axis=0,
    )
    return np.ascontiguousarray(
        outP.reshape(B, SIZE, SIZE).transpose(0, 2, 1).reshape(B, N)
    )
